# revision 7
# baseline (speedup 1.0000x reference)
"""CalibCLIP ViT encoder on 8 Trainium2 NeuronCores — pure data parallel.

Shards the 64-image batch as 8 images per core; CLIP weights replicated.
Per core: patch-embed matmul + 12 transformer layers + projection head,
all matmuls in bf16 with fp32 PSUM accumulation, LN/softmax/residual in fp32.

Layout strategy per image:
  - residual x: [SEQ=193, D=768] fp32 as two partition tiles [128,768]+[65,768]
  - LN on the natural layout (bn_stats over free dim), gains/biases folded into
    the following matmul weights on the host
  - matmul inputs: xn transposed to [D, SEQ] via PE transpose (bf16), images
    processed in pairs so the moving operand is [*, 386]
  - attention computed per (image, head) in "scores-transposed" orientation
    [k, q]; softmax sums obtained via a ones-column appended to V; the
    normalization uses gpsimd partition_broadcast + DVE multiplies
  - attn-out and MLP second matmul run activation-stationary so the residual
    returns in [SEQ, D] without further transposes
"""

import numpy as np
import ml_dtypes

import concourse.bass as bass
import concourse.mybir as mybir
import concourse.tile as tile
from concourse import bacc
from concourse.bass_utils import run_bass_kernel_spmd
from concourse.masks import make_identity

F32 = mybir.dt.float32
BF16 = mybir.dt.bfloat16
ACTF = mybir.ActivationFunctionType
NPBF16 = ml_dtypes.bfloat16

B, D, L, NH, DFF, OUT = 64, 768, 12, 12, 3072, 512
SEQ, NPATCH, DH = 193, 192, 64
KT = D // 128            # 6 k-tiles over D
DKT = DFF // 128         # 24 k-tiles over DFF
NCORES = 8
IPC = B // NCORES        # images per core
NPAIR = IPC // 2
# per-image partition tiling of SEQ: [128, 65]
MT = ((0, 128, 0), (1, 65, 128))   # (tile idx, rows, row offset)


def _build_nc(flags):
    nz_bqk, nz_bv, nz_bo, nz_b1, nz_b2, nz_bproj, use_lnpre_gb = flags
    nc = bacc.Bacc("TRN2", target_bir_lowering=False)

    pt_d = nc.declare_dram_parameter("pt", [IPC, D, NPATCH], BF16, isOutput=False)
    wc_d = nc.declare_dram_parameter("wc", [D, D], BF16, isOutput=False)
    wqk_d = nc.declare_dram_parameter("wqk", [L, D, 2 * D], BF16, isOutput=False)
    wv_d = nc.declare_dram_parameter("wv", [L, D, D], BF16, isOutput=False)
    wo_d = nc.declare_dram_parameter("wo", [L, D, D], BF16, isOutput=False)
    w1_d = nc.declare_dram_parameter("w1", [L, D, DFF], BF16, isOutput=False)
    w2_d = nc.declare_dram_parameter("w2", [L, DFF, D], BF16, isOutput=False)
    proj_d = nc.declare_dram_parameter("proj", [D, OUT], BF16, isOutput=False)
    pos_d = nc.declare_dram_parameter("pos_patch", [NPATCH, D], F32, isOutput=False)
    cls0_d = nc.declare_dram_parameter("cls_pos0", [1, D], F32, isOutput=False)
    bqk_d = bv_d = bo_d = b1_d = b2_d = bproj_d = None
    if nz_bqk:
        bqk_d = nc.declare_dram_parameter("bqk", [L, 2 * D], BF16, isOutput=False)
    if nz_bv:
        bv_d = nc.declare_dram_parameter("bv", [L, D], BF16, isOutput=False)
    if nz_bo:
        bo_d = nc.declare_dram_parameter("bo", [L, D], BF16, isOutput=False)
    if nz_b1:
        b1_d = nc.declare_dram_parameter("b1", [L, DFF], BF16, isOutput=False)
    if nz_b2:
        b2_d = nc.declare_dram_parameter("b2", [L, D], BF16, isOutput=False)
    if nz_bproj:
        bproj_d = nc.declare_dram_parameter("bproj", [1, OUT], BF16, isOutput=False)
    lnpre_g_d = lnpre_b_d = None
    if use_lnpre_gb:
        lnpre_g_d = nc.declare_dram_parameter("lnpre_g", [1, D], F32, isOutput=False)
        lnpre_b_d = nc.declare_dram_parameter("lnpre_b", [1, D], F32, isOutput=False)

    feats_d = nc.declare_dram_parameter("feats", [IPC, OUT], F32, isOutput=True)
    catt_d = nc.declare_dram_parameter("catt", [IPC, NPATCH], F32, isOutput=True)

    with tile.TileContext(nc) as tc:
        import contextlib
        ctx = contextlib.ExitStack()
        with ctx:
            singles = ctx.enter_context(tc.tile_pool(name="singles", bufs=1))
            xpool = ctx.enter_context(tc.tile_pool(name="xpool", bufs=1))
            wbig = ctx.enter_context(tc.tile_pool(name="wbig", bufs=1))
            wsm = ctx.enter_context(tc.tile_pool(name="wsm", bufs=6))
            actp = ctx.enter_context(tc.tile_pool(name="actp", bufs=2))
            lnp = ctx.enter_context(tc.tile_pool(name="lnp", bufs=3))
            qkp = ctx.enter_context(tc.tile_pool(name="qkp", bufs=1))
            vop = ctx.enter_context(tc.tile_pool(name="vop", bufs=1))
            expp = ctx.enter_context(tc.tile_pool(name="expp", bufs=4))
            rsp = ctx.enter_context(tc.tile_pool(name="rsp", bufs=2))
            sigp = ctx.enter_context(tc.tile_pool(name="sigp", bufs=3))
            biasp = ctx.enter_context(tc.tile_pool(name="biasp", bufs=2))
            hpool = ctx.enter_context(tc.tile_pool(name="hpool", bufs=1))
            # PSUM: 8 banks total: pmm 2 + psc 2 + pav 3 + ptr 1
            pmm = ctx.enter_context(tc.tile_pool(name="pmm", bufs=2, space="PSUM"))
            pscp = ctx.enter_context(tc.tile_pool(name="pscp", bufs=2, space="PSUM"))
            pavp = ctx.enter_context(tc.tile_pool(name="pavp", bufs=3, space="PSUM"))
            ptrp = ctx.enter_context(tc.tile_pool(name="ptrp", bufs=1, space="PSUM"))

            # ---- persistent constants ----
            ident = singles.tile([128, 128], BF16)
            make_identity(nc, ident)
            eps_sb = singles.tile([128, 1], F32)
            nc.vector.memset(eps_sb, 1e-5)
            ones_row = singles.tile([1, OUT], BF16)
            nc.vector.memset(ones_row, 1.0)

            # residual stream: x[img][tile]
            x_sb = [
                [xpool.tile([msz, D], F32, name=f"x_{i}_{t}") for (t, msz, _) in MT]
                for i in range(IPC)
            ]

            gbc = bbc = None
            if use_lnpre_gb:
                g_row = singles.tile([1, D], F32)
                b_row = singles.tile([1, D], F32)
                nc.sync.dma_start(out=g_row, in_=lnpre_g_d[:, :])
                nc.sync.dma_start(out=b_row, in_=lnpre_b_d[:, :])
                gbc = singles.tile([128, D], F32)
                bbc = singles.tile([128, D], F32)
                nc.gpsimd.partition_broadcast(gbc, g_row)
                nc.gpsimd.partition_broadcast(bbc, b_row)

            # transpose helper: PE-transpose [msz,128] blocks of src into dst
            tr_state = {"tile": None, "j": 0}

            def pe_transpose(dst_ap, src_ap, msz):
                if tr_state["tile"] is None or tr_state["j"] == 8:
                    tr_state["tile"] = ptrp.tile([128, 8, 128], BF16, name="trps")
                    tr_state["j"] = 0
                j = tr_state["j"]
                tr_state["j"] += 1
                tp = tr_state["tile"]
                nc.tensor.transpose(tp[:, j, 0:msz], src_ap, ident[0:msz, 0:msz])
                nc.scalar.copy(out=dst_ap, in_=tp[:, j, 0:msz])

            # LayerNorm stats + normalize into out tiles (optionally in-place f32)
            def ln_tiles(xt_list, out_list, apply_gb=False):
                for (t, msz, _) in MT:
                    xt = xt_list[t]
                    stats = lnp.tile([128, 3, 6], F32, name="lnstats")
                    for g in range(3):
                        nc.vector.bn_stats(
                            out=stats[0:msz, g, :],
                            in_=xt[0:msz, g * 256:(g + 1) * 256],
                        )
                    mv = lnp.tile([128, 2], F32, name="lnmv")
                    nc.vector.bn_aggr(out=mv[0:msz], in_=stats[0:msz])
                    std = lnp.tile([128, 1], F32, name="lnstd")
                    nc.scalar.activation(
                        out=std[0:msz], in_=mv[0:msz, 1:2], func=ACTF.Sqrt,
                        bias=eps_sb[0:msz],
                    )
                    rstd = lnp.tile([128, 1], F32, name="lnrstd")
                    nc.vector.reciprocal(rstd[0:msz], std[0:msz])
                    nc.vector.tensor_scalar(
                        out=out_list[t][0:msz, :], in0=xt[0:msz, :],
                        scalar1=mv[0:msz, 0:1], scalar2=rstd[0:msz],
                        op0=mybir.AluOpType.subtract, op1=mybir.AluOpType.mult,
                    )
                    if apply_gb:
                        nc.vector.tensor_mul(
                            out_list[t][0:msz, :], out_list[t][0:msz, :], gbc[0:msz, :])
                        nc.vector.tensor_add(
                            out_list[t][0:msz, :], out_list[t][0:msz, :], bbc[0:msz, :])

            # LN a pair of images -> bf16 -> PE-transpose into xnT[d] [128, 386]
            def ln_transpose_pair(pair):
                xnT = [actp.tile([128, 2 * SEQ], BF16, name=f"xnT_{d}") for d in range(KT)]
                for s in range(2):
                    i = 2 * pair + s
                    xn = [lnp.tile([msz, D], BF16, name=f"xn_{t}") for (t, msz, _) in MT]
                    ln_tiles(x_sb[i], xn)
                    for d in range(KT):
                        for (t, msz, roff) in MT:
                            pe_transpose(
                                xnT[d][:, s * SEQ + roff: s * SEQ + roff + msz],
                                xn[t][0:msz, d * 128:(d + 1) * 128],
                                msz,
                            )
                return xnT

            # ---------------- patch embedding ----------------
            wc_sb = wbig.tile([128, KT, D], BF16, name="wv_sb")
            for k in range(KT):
                nc.sync.dma_start(out=wc_sb[:, k, :], in_=wc_d[k * 128:(k + 1) * 128, :])
            pos_sb = [singles.tile([msz, D], F32, name=f"pos_{t}") for (t, msz, _) in ((0, 128, 0), (1, 64, 128))]
            nc.sync.dma_start(out=pos_sb[0], in_=pos_d[0:128, :])
            nc.sync.dma_start(out=pos_sb[1], in_=pos_d[128:192, :])
            for i in range(IPC):
                pt_sb = actp.tile([128, KT, NPATCH], BF16, name="pt_sb")
                for k in range(KT):
                    nc.sync.dma_start(
                        out=pt_sb[:, k, :], in_=pt_d[i, k * 128:(k + 1) * 128, :])
                tmp = [singles.tile([msz, D], F32, name=f"etmp_{t}") for (t, msz) in ((0, 128), (1, 64))]
                for (mt, msz) in ((0, 128), (1, 64)):
                    for nch in range(2):
                        ps = pmm.tile([128, OUT], F32, name="pmm")
                        for k in range(KT):
                            nc.tensor.matmul(
                                ps[0:msz, 0:384],
                                pt_sb[:, k, mt * 128: mt * 128 + msz],
                                wc_sb[:, k, nch * 384:(nch + 1) * 384],
                                start=(k == 0), stop=(k == KT - 1),
                            )
                        nc.vector.tensor_add(
                            tmp[mt][0:msz, nch * 384:(nch + 1) * 384],
                            ps[0:msz, 0:384],
                            pos_sb[mt][0:msz, nch * 384:(nch + 1) * 384],
                        )
                # scatter into x with +1 row shift (cls at row 0)
                nc.sync.dma_start(out=x_sb[i][0][0:1, :], in_=cls0_d[:, :])
                nc.sync.dma_start(out=x_sb[i][0][1:128, :], in_=tmp[0][0:127, :])
                nc.sync.dma_start(out=x_sb[i][1][0:1, :], in_=tmp[0][127:128, :])
                nc.sync.dma_start(out=x_sb[i][1][1:65, :], in_=tmp[1][0:64, :])
                # ln_pre in place
                ln_tiles(x_sb[i], x_sb[i], apply_gb=use_lnpre_gb)

            # ---------------- transformer layers ----------------
            for l in range(L):
                wqk_sb = wbig.tile([128, KT, 2 * D], BF16, name="wqk_sb")
                wv_sb = wbig.tile([128, KT, D], BF16, name="wv_sb")
                wo_sb = wbig.tile([128, KT, D], BF16, name="wo_sb")
                for k in range(KT):
                    nc.sync.dma_start(out=wqk_sb[:, k, :], in_=wqk_d[l, k * 128:(k + 1) * 128, :])
                    nc.sync.dma_start(out=wv_sb[:, k, :], in_=wv_d[l, k * 128:(k + 1) * 128, :])
                    nc.sync.dma_start(out=wo_sb[:, k, :], in_=wo_d[l, k * 128:(k + 1) * 128, :])
                bias_sb = {}
                for key, dram, width in (
                    ("bqk", bqk_d, 2 * D), ("bv", bv_d, D), ("bo", bo_d, D),
                    ("b1", b1_d, DFF), ("b2", b2_d, D),
                ):
                    if dram is not None:
                        bias_sb[key] = biasp.tile([1, width], BF16, name=f"{key}_sb")
                        nc.sync.dma_start(out=bias_sb[key], in_=dram[l:l + 1, :])

                for pair in range(NPAIR):
                    xnT = ln_transpose_pair(pair)

                    # ---- qkv: q,k weight-stationary (transposed out) ----
                    qkT = [qkp.tile([128, 2 * SEQ], BF16, name=f"qkT_{m}") for m in range(12)]
                    for m in range(12):
                        ps = pmm.tile([128, OUT], F32, name="pmm")
                        for k in range(KT):
                            nc.tensor.matmul(
                                ps[:, 0:2 * SEQ], wqk_sb[:, k, m * 128:(m + 1) * 128],
                                xnT[k][:, :], start=(k == 0),
                                stop=(k == KT - 1 and "bqk" not in bias_sb),
                            )
                        if "bqk" in bias_sb:
                            nc.tensor.matmul(
                                ps[:, 0:2 * SEQ], bias_sb["bqk"][0:1, m * 128:(m + 1) * 128],
                                ones_row[0:1, 0:2 * SEQ], start=False, stop=True)
                        nc.scalar.copy(out=qkT[m][:, :], in_=ps[:, 0:2 * SEQ])
                    qT, kT = qkT[:6], qkT[6:]

                    # ---- v activation-stationary with ones column ----
                    vaug = [
                        [vop.tile([msz, NH, DH + 1], BF16, name=f"vaug_{s}_{t}")
                         for (t, msz, _) in MT]
                        for s in range(2)
                    ]
                    for s in range(2):
                        for (t, msz, roff) in MT:
                            nc.vector.memset(vaug[s][t][0:msz, :, DH:DH + 1], 1.0)
                            for nch in range(2):
                                ps = pmm.tile([128, OUT], F32, name="pmm")
                                for k in range(KT):
                                    nc.tensor.matmul(
                                        ps[0:msz, 0:384],
                                        xnT[k][:, s * SEQ + roff: s * SEQ + roff + msz],
                                        wv_sb[:, k, nch * 384:(nch + 1) * 384],
                                        start=(k == 0),
                                        stop=(k == KT - 1 and "bv" not in bias_sb),
                                    )
                                if "bv" in bias_sb:
                                    nc.tensor.matmul(
                                        ps[0:msz, 0:384], ones_row[0:1, 0:msz],
                                        bias_sb["bv"][0:1, nch * 384:(nch + 1) * 384],
                                        start=False, stop=True)
                                nc.scalar.copy(
                                    out=vaug[s][t][0:msz, nch * 6:(nch + 1) * 6, 0:DH],
                                    in_=ps[0:msz, 0:384])

                    # ---- attention (per image; heads in waves of 6) ----
                    outT = [vop.tile([128, 2 * SEQ], BF16, name=f"outT_{d}") for d in range(KT)]
                    for s in range(2):
                        soff = s * SEQ
                        for wave in range(2):
                            pavs = []
                            for tt in range(3):
                                dt_ = wave * 3 + tt
                                pav = pavp.tile([DH + 1, 2 * SEQ], F32, name="pav")
                                pavs.append(pav)
                                for ss in range(2):
                                    h = 2 * dt_ + ss
                                    hp, hr = h // 2, (h % 2) * 64
                                    psc = pscp.tile([128, 2, SEQ], F32, name="psc")
                                    nc.tensor.matmul(
                                        psc[:, 0, :], kT[hp][hr:hr + 64, soff:soff + 128],
                                        qT[hp][hr:hr + 64, soff:soff + SEQ],
                                        start=True, stop=True)
                                    nc.tensor.matmul(
                                        psc[0:65, 1, :], kT[hp][hr:hr + 64, soff + 128:soff + SEQ],
                                        qT[hp][hr:hr + 64, soff:soff + SEQ],
                                        start=True, stop=True)
                                    e0 = expp.tile([128, SEQ], BF16, name="expT_0")
                                    e1 = expp.tile([65, SEQ], BF16, name="expT_1")
                                    nc.scalar.activation(out=e0, in_=psc[:, 0, :], func=ACTF.Exp, scale=0.125)
                                    nc.scalar.activation(out=e1[0:65, :], in_=psc[0:65, 1, :], func=ACTF.Exp, scale=0.125)
                                    nc.tensor.matmul(
                                        pav[:, ss * SEQ:(ss + 1) * SEQ],
                                        vaug[s][0][:, h, :], e0, start=True, stop=False)
                                    nc.tensor.matmul(
                                        pav[:, ss * SEQ:(ss + 1) * SEQ],
                                        vaug[s][1][0:65, h, :], e1[0:65, :],
                                        start=False, stop=True)
                            for tt in range(3):
                                dt_ = wave * 3 + tt
                                pav = pavs[tt]
                                rsum = rsp.tile([1, 2 * SEQ], F32, name="rsum")
                                nc.vector.reciprocal(rsum, pav[DH:DH + 1, :])
                                for ss in range(2):
                                    rbt = rsp.tile([64, SEQ], F32, name=f"rbt_{ss}")
                                    nc.gpsimd.partition_broadcast(
                                        rbt, rsum[0:1, ss * SEQ:(ss + 1) * SEQ])
                                    nc.vector.tensor_mul(
                                        outT[dt_][ss * 64:(ss + 1) * 64, soff:soff + SEQ],
                                        pav[0:64, ss * SEQ:(ss + 1) * SEQ],
                                        rbt[0:64, :])

                        # ---- cls attention row (last layer only) ----
                        if l == L - 1:
                            i = 2 * pair + s
                            attm = singles.tile([NH, SEQ], F32, name="attm")
                            for h in range(NH):
                                hp, hr = h // 2, (h % 2) * 64
                                psc = pscp.tile([128, 2, SEQ], F32, name="psc")
                                nc.tensor.matmul(
                                    psc[0:1, 0, :], qT[hp][hr:hr + 64, soff:soff + 1],
                                    kT[hp][hr:hr + 64, soff:soff + SEQ],
                                    start=True, stop=True)
                                arow = rsp.tile([1, SEQ], F32, name="arow")
                                nc.scalar.activation(
                                    out=arow, in_=psc[0:1, 0, :],
                                    func=ACTF.Exp, scale=0.125)
                                nc.sync.dma_start(out=attm[h:h + 1, :], in_=arow)
                            asum = singles.tile([NH, 1], F32, name="asum")
                            nc.vector.reduce_sum(out=asum, in_=attm, axis=mybir.AxisListType.X)
                            arcp = singles.tile([NH, 1], F32, name="arcp")
                            nc.vector.reciprocal(arcp, asum)
                            psa = pscp.tile([128, 2, SEQ], F32, name="psc")
                            nc.tensor.matmul(psa[0:1, 0, :], arcp, attm, start=True, stop=True)
                            cat = singles.tile([1, SEQ], F32, name="cat")
                            nc.scalar.mul(out=cat, in_=psa[0:1, 0, :], mul=1.0 / NH)
                            nc.sync.dma_start(out=catt_d[i:i + 1, :], in_=cat[0:1, 1:SEQ])

                    # ---- attn proj (activation-stationary) + residual ----
                    for s in range(2):
                        i = 2 * pair + s
                        for (t, msz, roff) in MT:
                            for nch in range(2):
                                ps = pmm.tile([128, OUT], F32, name="pmm")
                                for k in range(KT):
                                    nc.tensor.matmul(
                                        ps[0:msz, 0:384],
                                        outT[k][:, s * SEQ + roff: s * SEQ + roff + msz],
                                        wo_sb[:, k, nch * 384:(nch + 1) * 384],
                                        start=(k == 0),
                                        stop=(k == KT - 1 and "bo" not in bias_sb),
                                    )
                                if "bo" in bias_sb:
                                    nc.tensor.matmul(
                                        ps[0:msz, 0:384], ones_row[0:1, 0:msz],
                                        bias_sb["bo"][0:1, nch * 384:(nch + 1) * 384],
                                        start=False, stop=True)
                                nc.vector.tensor_add(
                                    x_sb[i][t][0:msz, nch * 384:(nch + 1) * 384],
                                    x_sb[i][t][0:msz, nch * 384:(nch + 1) * 384],
                                    ps[0:msz, 0:384])

                    # ---- MLP ----
                    xnT2 = ln_transpose_pair(pair)
                    hT = [hpool.tile([128, 2 * SEQ], BF16, name=f"hT_{m}") for m in range(DKT)]
                    for m in range(DKT):
                        ps = pmm.tile([128, OUT], F32, name="pmm")
                        for k in range(KT):
                            w1t = wsm.tile([128, 128], BF16, name="w1t")
                            nc.sync.dma_start(
                                out=w1t,
                                in_=w1_d[l, k * 128:(k + 1) * 128, m * 128:(m + 1) * 128])
                            nc.tensor.matmul(
                                ps[:, 0:2 * SEQ], w1t, xnT2[k][:, :],
                                start=(k == 0),
                                stop=(k == KT - 1 and "b1" not in bias_sb))
                        if "b1" in bias_sb:
                            nc.tensor.matmul(
                                ps[:, 0:2 * SEQ], bias_sb["b1"][0:1, m * 128:(m + 1) * 128],
                                ones_row[0:1, 0:2 * SEQ], start=False, stop=True)
                        sig = sigp.tile([128, 2 * SEQ], F32, name="sig")
                        nc.scalar.activation(out=sig, in_=ps[:, 0:2 * SEQ], func=ACTF.Sigmoid, scale=1.702)
                        nc.vector.tensor_mul(hT[m][:, :], ps[:, 0:2 * SEQ], sig)
                    # w2 activation-stationary + residual; two (s,t) groups share
                    # each streamed w2 tile
                    for nch in range(2):
                        for grp in range(2):
                            combos = [(s, mt) for s in range(2) for mt in range(2)][grp * 2:(grp + 1) * 2]
                            pss = {}
                            for (s, mt) in combos:
                                pss[(s, mt)] = pmm.tile([128, OUT], F32, name="pmm")
                            for k in range(DKT):
                                w2t = wsm.tile([128, 384], BF16, name="w2t")
                                nc.sync.dma_start(
                                    out=w2t,
                                    in_=w2_d[l, k * 128:(k + 1) * 128, nch * 384:(nch + 1) * 384])
                                for (s, mt) in combos:
                                    msz, roff = MT[mt][1], MT[mt][2]
                                    nc.tensor.matmul(
                                        pss[(s, mt)][0:msz, 0:384],
                                        hT[k][:, s * SEQ + roff: s * SEQ + roff + msz],
                                        w2t, start=(k == 0),
                                        stop=(k == DKT - 1 and "b2" not in bias_sb))
                            for (s, mt) in combos:
                                msz = MT[mt][1]
                                if "b2" in bias_sb:
                                    nc.tensor.matmul(
                                        pss[(s, mt)][0:msz, 0:384], ones_row[0:1, 0:msz],
                                        bias_sb["b2"][0:1, nch * 384:(nch + 1) * 384],
                                        start=False, stop=True)
                                i = 2 * pair + s
                                nc.vector.tensor_add(
                                    x_sb[i][mt][0:msz, nch * 384:(nch + 1) * 384],
                                    x_sb[i][mt][0:msz, nch * 384:(nch + 1) * 384],
                                    pss[(s, mt)][0:msz, 0:384])

            # ---------------- head: ln_post + projection + L2 norm ----------------
            proj_sb = wbig.tile([128, KT, OUT], BF16, name="wv_sb")
            for k in range(KT):
                nc.sync.dma_start(out=proj_sb[:, k, :], in_=proj_d[k * 128:(k + 1) * 128, :])
            clsm = singles.tile([IPC, D], BF16)
            for i in range(IPC):
                xn = [lnp.tile([msz, D], BF16, name=f"xn_{t}") for (t, msz, _) in MT]
                ln_tiles(x_sb[i], xn)
                nc.sync.dma_start(out=clsm[i:i + 1, :], in_=xn[0][0:1, :])
            clsmT = singles.tile([128, KT, IPC], BF16)
            for k in range(KT):
                pe_transpose(clsmT[:, k, :], clsm[0:IPC, k * 128:(k + 1) * 128], IPC)
            psf = pmm.tile([128, OUT], F32, name="pmm")
            for k in range(KT):
                nc.tensor.matmul(
                    psf[0:IPC, :], clsmT[:, k, :], proj_sb[:, k, :],
                    start=(k == 0), stop=(k == KT - 1 and bproj_d is None))
            if bproj_d is not None:
                bproj_sb = singles.tile([1, OUT], BF16)
                nc.sync.dma_start(out=bproj_sb, in_=bproj_d[:, :])
                nc.tensor.matmul(
                    psf[0:IPC, :], ones_row[0:1, 0:IPC], bproj_sb,
                    start=False, stop=True)
            sq = singles.tile([IPC, OUT], F32, name="sq")
            nc.scalar.activation(out=sq, in_=psf[0:IPC, :], func=ACTF.Square)
            ssum = singles.tile([IPC, 1], F32, name="ssum")
            nc.vector.reduce_sum(out=ssum, in_=sq, axis=mybir.AxisListType.X)
            sroot = singles.tile([IPC, 1], F32, name="sroot")
            nc.scalar.activation(out=sroot, in_=ssum, func=ACTF.Sqrt)
            rs = singles.tile([IPC, 1], F32, name="rs")
            nc.vector.reciprocal(rs, sroot)
            feats_sb = singles.tile([IPC, OUT], F32, name="feats_sb")
            nc.vector.tensor_scalar_mul(feats_sb, in0=psf[0:IPC, :], scalar1=rs)
            nc.sync.dma_start(out=feats_d[:, :], in_=feats_sb)

    nc.finalize()
    return nc


_NC_CACHE = {}


def _get_nc(flags):
    if flags not in _NC_CACHE:
        _NC_CACHE[flags] = _build_nc(flags)
    return _NC_CACHE[flags]


def _prepare(images, params):
    p = {k: np.asarray(v, dtype=np.float32) for k, v in params.items()}
    images = np.asarray(images, dtype=np.float32)

    w_qkv_eff = p["ln1_g"][:, :, None] * p["w_qkv"]
    b_qkv_eff = p["b_qkv"] + np.einsum("ld,ldf->lf", p["ln1_b"], p["w_qkv"])
    w1_eff = p["ln2_g"][:, :, None] * p["w1"]
    b1_eff = p["b1"] + np.einsum("ld,ldf->lf", p["ln2_b"], p["w1"])
    proj_eff = p["ln_post_g"][:, None] * p["proj"]
    b_proj = p["ln_post_b"] @ p["proj"]

    patches = (
        images.reshape(B, 3, 24, 16, 8, 16)
        .transpose(0, 2, 4, 3, 5, 1)
        .reshape(B, NPATCH, D)
    )
    patchesT = np.ascontiguousarray(patches.transpose(0, 2, 1)).astype(NPBF16)

    lnpre_identity = bool(
        np.all(p["ln_pre_g"] == 1.0) and np.all(p["ln_pre_b"] == 0.0))
    flags = (
        bool(np.any(b_qkv_eff[:, :2 * D])), bool(np.any(b_qkv_eff[:, 2 * D:])),
        bool(np.any(p["b_o"])), bool(np.any(b1_eff)), bool(np.any(p["b2"])),
        bool(np.any(b_proj)), not lnpre_identity,
    )

    common = {
        "wc": np.ascontiguousarray(p["conv_w"].reshape(D, D)).astype(NPBF16),
        "wqk": np.ascontiguousarray(w_qkv_eff[:, :, :2 * D]).astype(NPBF16),
        "wv": np.ascontiguousarray(w_qkv_eff[:, :, 2 * D:]).astype(NPBF16),
        "wo": p["w_o"].astype(NPBF16),
        "w1": w1_eff.astype(NPBF16),
        "w2": p["w2"].astype(NPBF16),
        "proj": proj_eff.astype(NPBF16),
        "pos_patch": np.ascontiguousarray(p["pos"][1:]).astype(np.float32),
        "cls_pos0": (p["cls"] + p["pos"][0])[None, :].astype(np.float32),
    }
    nz_bqk, nz_bv, nz_bo, nz_b1, nz_b2, nz_bproj, use_lnpre = flags
    if nz_bqk:
        common["bqk"] = np.ascontiguousarray(b_qkv_eff[:, :2 * D]).astype(NPBF16)
    if nz_bv:
        common["bv"] = np.ascontiguousarray(b_qkv_eff[:, 2 * D:]).astype(NPBF16)
    if nz_bo:
        common["bo"] = p["b_o"].astype(NPBF16)
    if nz_b1:
        common["b1"] = b1_eff.astype(NPBF16)
    if nz_b2:
        common["b2"] = p["b2"].astype(NPBF16)
    if nz_bproj:
        common["bproj"] = b_proj[None, :].astype(NPBF16)
    if use_lnpre:
        common["lnpre_g"] = p["ln_pre_g"][None, :].astype(np.float32)
        common["lnpre_b"] = p["ln_pre_b"][None, :].astype(np.float32)

    in_maps = []
    for c in range(NCORES):
        m = dict(common)
        m["pt"] = patchesT[c * IPC:(c + 1) * IPC]
        in_maps.append(m)
    return flags, in_maps


def make_runner(images, params):
    """Persistent runner: compile once, keep inputs device-resident, call many
    times. Returns (call_fn, flags). call_fn() -> (feats[64,512], catt[64,192])."""
    import jax
    from jax.experimental.shard_map import shard_map
    from jax.sharding import Mesh, PartitionSpec, NamedSharding
    from concourse import bass2jax

    flags, in_maps = _prepare(images, params)
    nc = _get_nc(flags)
    call = _make_caller(nc, in_maps)

    def call2():
        om = call()
        return om["feats"], om["catt"]

    return call2, flags


def _make_caller(nc, in_maps):
    import jax
    from jax.experimental.shard_map import shard_map
    from jax.sharding import Mesh, PartitionSpec, NamedSharding
    from concourse import bass2jax
    bass2jax.install_neuronx_cc_hook()

    part_name = nc.partition_id_tensor.name if nc.partition_id_tensor else None
    in_names, out_names, out_avals, zero_shapes = [], [], [], []
    for alloc in nc.m.functions[0].allocations:
        if not isinstance(alloc, mybir.MemoryLocationSet):
            continue
        name = alloc.memorylocations[0].name
        if alloc.kind == "ExternalInput":
            if name != part_name:
                in_names.append(name)
        elif alloc.kind == "ExternalOutput":
            out_names.append(name)
            shape = tuple(alloc.tensor_shape)
            dtype = mybir.dt.np(alloc.dtype)
            out_avals.append(jax.core.ShapedArray(shape, dtype))
            zero_shapes.append((shape, dtype))
    n_params = len(in_names)
    n_outs = len(out_names)
    all_names = in_names + out_names
    if part_name is not None:
        all_names = all_names + [part_name]

    def _body(*args):
        operands = list(args)
        if part_name is not None:
            operands.append(bass2jax.partition_id_tensor())
        outs = bass2jax._bass_exec_p.bind(
            *operands,
            out_avals=tuple(out_avals),
            in_names=tuple(all_names),
            out_names=tuple(out_names),
            lowering_input_output_aliases=(),
            sim_require_finite=True,
            sim_require_nnan=True,
            nc=nc,
        )
        return tuple(outs)

    devices = jax.devices()[:NCORES]
    mesh = Mesh(np.asarray(devices), ("core",))
    spec = PartitionSpec("core")
    in_specs = (spec,) * (n_params + n_outs)
    out_specs = (spec,) * n_outs
    donate = tuple(range(n_params, n_params + n_outs))
    sharded = jax.jit(
        shard_map(_body, mesh=mesh, in_specs=in_specs, out_specs=out_specs,
                  check_rep=False),
        donate_argnums=donate, keep_unused=True)
    sh = NamedSharding(mesh, spec)
    in_dev = [
        jax.device_put(
            np.concatenate([np.asarray(m[name]) for m in in_maps], axis=0), sh)
        for name in in_names
    ]

    def call():
        zeros = [jax.device_put(np.zeros((NCORES * s0[0], *s0[1:]), dt), sh)
                 for (s0, dt) in zero_shapes]
        outs = sharded(*in_dev, *zeros)
        jax.block_until_ready(outs)
        return {name: np.asarray(outs[i]) for i, name in enumerate(out_names)}

    return call


def run_device(images, params, trace=False, **kw):
    flags, in_maps = _prepare(images, params)
    nc = _get_nc(flags)
    res = run_bass_kernel_spmd(nc, in_maps, list(range(NCORES)), trace=trace, **kw)
    feats = np.concatenate([r["feats"] for r in res.results], axis=0)
    catt = np.concatenate([r["catt"] for r in res.results], axis=0)
    return (feats, catt), res


def kernel(images, params):
    (feats, catt), _ = run_device(images, params, trace=False)
    return feats, catt
